# revision 28
# baseline (speedup 1.0000x reference)
"""MLA (absorbed-weight multi-head latent attention) TRN2 Bass kernel. v5

Problem: B=2, N=NKV=2048, E=4096, H=16, HD=256, LQ=512, LKV=256.
  C_q  = Q @ Wq_d                 [B,N,LQ]
  C_kv = K @ Wkv_d                [B,Nkv,LKV]
  CqWqk = (C_q @ W_qk)            [B,N,H,LKV]
  scores = einsum('bnhl,bkl->bhnk', CqWqk, C_kv) / sqrt(LKV)
  attn = softmax(scores, -1)
  V_up = (C_kv @ Wv_u)            [B,Nkv,H,HD]
  out  = einsum('bhnk,bkhd->bnhd', attn, V_up) -> [B,N,E]

Sharding: 8 cores = (batch b in 0..1) x (query quarter q in 0..3).
Each core handles n-rows [q*512,(q+1)*512) of batch b for ALL heads.

Structure (all matmul operands bf16, PSUM f32, per-core):
  phase 1: C_kvT [LKV, k]  = lhsT Wkv_d @ rhs K^T   (contract E, streamed KT)
  phase 2: C_qT  [LQ, n]   = lhsT Wq_d  @ rhs Q^T   (contract E, streamed QT)
           ckv_aug [k, LKV|1 1] = XBAR dma-transpose of C_kvT (+ ones cols)
  head pipeline (software-pipelined 3 deep; head h's loop interleaves):
    CqWqkT_h [LKV, n] = lhsT W_qk_h @ rhs C_qT      (contract LQ)
    S^T(h)   [k, n]   = lhsT C_kvT  @ rhs CqWqkT_h  (contract LKV)
    P^T(h)   = exp(S^T / 16) -> persistent pt tiles (16 per head; the
               consumer is a full head behind, so ACT latency never stalls PE)
    PV(h-1): O_lat [n, LKV|den] = lhsT P^T @ rhs ckv_aug  (contract k,
               nk-major so only 2 rotating PSUM banks are needed)
    onorm(h-1) = O_lat[:, :256] * recip(den); XBAR-transpose -> onormT
    upproj(h-2): out [n, HD] = lhsT onormT @ rhs Wv_u_h   (contract LKV)
"""
import numpy as np

B, N, NKV, E, H = 2, 2048, 2048, 4096, 16
HD, LQ, LKV = 256, 512, 256
NCORES = 8
NQ = N // 4          # 512 query rows per core
ECH = E // 128       # 32 e-chunks
KCH = NKV // 128     # 16 k-chunks
NCK = NQ // 128      # 4 n-chunks per core

_cache = {}


def build_nc(iters=1, stop_after="full", fake_ckv=False):
    import concourse.bass as bass
    from concourse import bacc
    import concourse.mybir as mybir
    import concourse.tile as tile

    dt = mybir.dt
    bf = dt.bfloat16
    f32 = dt.float32

    nc = bacc.Bacc(None, target_bir_lowering=False)
    QT = nc.dram_tensor("QT", [E, NQ], bf, kind="ExternalInput")
    KT = nc.dram_tensor("KT", [E, NKV], bf, kind="ExternalInput")
    WQD = nc.dram_tensor("WQD", [E, LQ], bf, kind="ExternalInput")
    WQK = nc.dram_tensor("WQK", [LQ, H * LKV], bf, kind="ExternalInput")
    WKVD = nc.dram_tensor("WKVD", [E, LKV], bf, kind="ExternalInput")
    WVU = nc.dram_tensor("WVU", [LKV, H * HD], bf, kind="ExternalInput")
    OUT = nc.dram_tensor("OUT", [NQ, E], bf, kind="ExternalOutput")

    Exp = mybir.ActivationFunctionType.Exp

    with tile.TileContext(nc) as tc:
        with tc.tile_pool(name="persist", bufs=1) as persist, \
             tc.tile_pool(name="psA", bufs=2, space="PSUM") as psA, \
             tc.tile_pool(name="psS", bufs=3, space="PSUM") as psS, \
             tc.tile_pool(name="psX", bufs=1, space="PSUM") as psX, \
             tc.tile_pool(name="qtp", bufs=5) as qtp, \
             tc.tile_pool(name="head", bufs=2) as hp, \
             tc.tile_pool(name="wvp", bufs=3) as wvp, \
             tc.tile_pool(name="ptp", bufs=32) as ptp, \
             tc.tile_pool(name="otp", bufs=3) as otp:
            loop_ctx = tc.For_i(0, iters, 1,
                                hint_engines=(mybir.EngineType.PE,)) \
                if iters > 1 else None
            if loop_ctx is not None:
                loop_ctx.__enter__()

            cqt = persist.tile([128, 4, NQ], bf)         # C_qT  [LQ, n]
            ckvt = persist.tile([128, 2, NKV], bf)       # C_kvT [LKV, k]
            ckva = persist.tile([128, KCH, 258], bf)     # [k, LKV | ones]

            def qt_prefetch(g):
                qte = qtp.tile([128, 4, NQ], bf, tag="qt", name=f"qt{g}")
                nc.sync.dma_start(
                    out=qte, in_=QT[g * 512:(g + 1) * 512, :]
                    .rearrange("(c p) n -> p c n", p=128))
                wqde = qtp.tile([128, 4, LQ], bf, tag="wqd", name=f"wqd{g}")
                nc.sync.dma_start(
                    out=wqde, in_=WQD[g * 512:(g + 1) * 512, :]
                    .rearrange("(c p) l -> p c l", p=128))
                return qte, wqde

            # ---------- phase 1: C_kvT over streamed KT ----------
            # 8 accumulators spread over every PSUM bank: o,o,v,v,sw,sw,sw,x
            qpre = {}
            with tc.tile_pool(name="ktp", bufs=4) as ktp, \
                 tc.tile_pool(name="wkp", bufs=2) as wkp:
                accs = [psA.tile([128, 512], f32, tag=t, name=f"acc{i}")
                        for i, t in enumerate(("o", "o", "v", "v"))]
                accs += [psS.tile([128, 512], f32, tag="sw", name=f"accs{i}")
                         for i in range(3)]
                accs.append(psX.tile([128, 512], f32, tag="x", name="accx"))
                for ec in range(ECH):
                    if ec % 4 == 0:
                        wkvd_t = wkp.tile([128, 4, LKV], bf, tag="wkvd")
                        if ec == 0:
                            nc.sync.dma_start(
                                out=wkvd_t[:, 0, :], in_=WKVD[0:128, :])
                            nc.sync.dma_start(
                                out=wkvd_t[:, 1:4, :],
                                in_=WKVD[128:512, :]
                                .rearrange("(c p) l -> p c l", p=128))
                        else:
                            nc.sync.dma_start(
                                out=wkvd_t,
                                in_=WKVD[ec * 128:(ec + 4) * 128, :]
                                .rearrange("(c p) l -> p c l", p=128))
                    ktt = ktp.tile([128, NKV], bf, tag="kt")
                    if ec == 0:
                        # halve the first chunk so the first matmul starts
                        # after ~256KB instead of 512KB of DMA
                        for half in range(2):
                            nc.sync.dma_start(
                                out=ktt[:, half * 1024:(half + 1) * 1024],
                                in_=KT[0:128, half * 1024:(half + 1) * 1024])
                    else:
                        nc.sync.dma_start(out=ktt,
                                          in_=KT[ec * 128:(ec + 1) * 128, :])
                    if ec in (16, 24):   # prefetch first C_q chunk groups
                        qpre[(ec - 16) // 8] = qt_prefetch((ec - 16) // 8)
                    for nt in range(4):
                        for lc in range(2):
                            nc.tensor.matmul(
                                accs[lc * 4 + nt],
                                wkvd_t[:, ec % 4, lc * 128:(lc + 1) * 128],
                                ktt[:, nt * 512:(nt + 1) * 512],
                                start=(ec == 0), stop=(ec == ECH - 1))
                for lc in range(2):
                    for nt in range(4):
                        dst = ckvt[:, lc, nt * 512:(nt + 1) * 512]
                        if nt % 2 == 0:
                            nc.vector.tensor_copy(dst, accs[lc * 4 + nt])
                        else:
                            nc.scalar.copy(dst, accs[lc * 4 + nt])

            # ckv_aug: XBAR-transpose C_kvT into [k, LKV] layout (+ones).
            # HW XBAR ignores strided out-APs: transpose to a contiguous tmp,
            # then DVE-copy into the strided ckva slice.
            nc.vector.memset(ckva[:, :, 256:258], 1.0)
            with tc.tile_pool(name="ckt", bufs=1) as ckt:
                for lkc in range(2):
                    tmp = ckt.tile([128, KCH, 128], bf, tag=f"ct{lkc}",
                                   name=f"ckvtr{lkc}")
                    nc.sync.dma_start(out=tmp, in_=ckvt[:, lkc, :],
                                      transpose=True)
                    nc.vector.tensor_copy(
                        ckva[:, :, lkc * 128:(lkc + 1) * 128], tmp)
            # prefetch head-0 weights
            wqk_h = hp.tile([128, 4, LKV], bf, tag="wqk")
            nc.sync.dma_start(
                out=wqk_h, in_=WQK[:, 0:LKV].rearrange("(c p) l -> p c l", p=128))
            wvu_d = {0: wvp.tile([128, 2, HD], bf, tag="wvu", name="wvu0")}
            nc.sync.dma_start(
                out=wvu_d[0],
                in_=WVU[:, 0:HD].rearrange("(c p) d -> p c d", p=128))

            # ---------- phase 2: C_qT over streamed QT/WQD (4-chunk DMAs) ----
            qps = [psA.tile([128, 512], f32, tag=t, name=f"qacc{i}")
                   for i, t in enumerate(("o", "o", "v", "v"))]
            for g in range(ECH // 4):
                qte, wqde = qpre[g] if g in qpre else qt_prefetch(g)
                for sub in range(4):
                    ec = g * 4 + sub
                    for lc in range(4):
                        nc.tensor.matmul(
                            qps[lc], wqde[:, sub, lc * 128:(lc + 1) * 128],
                            qte[:, sub, :],
                            start=(ec == 0), stop=(ec == ECH - 1))
            for lc in range(4):
                dst = cqt[:, lc, :]
                if lc % 2 == 0:
                    nc.vector.tensor_copy(dst, qps[lc])
                else:
                    nc.scalar.copy(dst, qps[lc])

            # ---------- phase 3: software-pipelined head loop ----------
            # During head h's score loop: PV+normalize of h-1, upproj of h-2.
            pts = {}      # h -> list of 16 pt tiles
            onTs = {}     # (h, nk) -> contiguous [128, 2, 128] onorm^T tile

            def emit_upproj(hh):
                wvu_prev = wvu_d.pop(hh)
                ot = otp.tile([128, NCK, HD], bf, tag="ot")
                for nk in range(NCK):
                    onT = onTs.pop((hh, nk))
                    psu = psA.tile([128, 256], f32, tag="v")
                    for lkc in range(2):
                        nc.tensor.matmul(
                            psu, onT[:, lkc, :], wvu_prev[:, lkc, :],
                            start=(lkc == 0), stop=(lkc == 1))
                    nc.vector.tensor_copy(ot[:, nk, :], psu)
                nc.sync.dma_start(
                    out=OUT.rearrange("(c p) e -> p c e", p=128)
                    [:, :, hh * HD:(hh + 1) * HD],
                    in_=ot)

            for h in range(H + 1):
                if h < H:
                    if h > 0:
                        wqk_h = hp.tile([128, 4, LKV], bf, tag="wqk")
                        nc.sync.dma_start(
                            out=wqk_h,
                            in_=WQK[:, h * LKV:(h + 1) * LKV]
                            .rearrange("(c p) l -> p c l", p=128))
                        wvu_d[h] = wvp.tile([128, 2, HD], bf, tag="wvu",
                                            name=f"wvu{h}")
                        nc.sync.dma_start(
                            out=wvu_d[h],
                            in_=WVU[:, h * HD:(h + 1) * HD]
                            .rearrange("(c p) d -> p c d", p=128))

                    # CqWqkT_h [2 x 128, n=512], contract LQ
                    cqwqk = hp.tile([128, 2, NQ], bf, tag="cqwqk")
                    for lkc in range(2):
                        ps = psS.tile([128, 512], f32, tag="sw")
                        for lc in range(4):
                            nc.tensor.matmul(
                                ps, wqk_h[:, lc, lkc * 128:(lkc + 1) * 128],
                                cqt[:, lc, :], start=(lc == 0), stop=(lc == 3))
                        nc.vector.tensor_copy(cqwqk[:, lkc, :], ps)

                if h >= 2:
                    emit_upproj(h - 2)

                if h >= 1:
                    onorm = hp.tile([128, NCK, LKV], bf, tag="onorm")
                    den = hp.tile([128, 4], f32, tag="den")
                if h == H:
                    otF = otp.tile([128, NCK, HD], bf, tag="ot", name="otF")

                    def emit_upproj_nk(nk):
                        onT = onTs.pop((H - 1, nk))
                        psu = psA.tile([128, 256], f32, tag="v",
                                       name=f"psuF{nk}")
                        for lkc in range(2):
                            nc.tensor.matmul(
                                psu, onT[:, lkc, :], wvu_d[H - 1][:, lkc, :],
                                start=(lkc == 0), stop=(lkc == 1))
                        nc.vector.tensor_copy(otF[:, nk, :], psu)

                for s in range(KCH):
                    if h < H:
                        # scores S^T(h, kc=s) + exp -> persistent pt
                        pss = psS.tile([128, 512], f32, tag="sw")
                        for lkc in range(2):
                            nc.tensor.matmul(
                                pss, ckvt[:, lkc, s * 128:(s + 1) * 128],
                                cqwqk[:, lkc, :],
                                start=(lkc == 0), stop=(lkc == 1))
                        pt = ptp.tile([128, NQ], bf, tag="pt",
                                      name=f"pt{h}_{s}")
                        pts.setdefault(h, []).append(pt)
                        nc.scalar.activation(out=pt, in_=pss, func=Exp,
                                             scale=1.0 / 16.0)

                    if h >= 1:
                        # latent PV of head h-1, nk-major: nk = s//4
                        nk, j = divmod(s, 4)
                        if j == 0:
                            pso = psA.tile([128, 258], f32, tag="o",
                                           name=f"pso{h - 1}_{nk}")
                        for kc in range(j * 4, j * 4 + 4):
                            nc.tensor.matmul(
                                pso, pts[h - 1][kc][:, nk * 128:(nk + 1) * 128],
                                ckva[:, kc, :],
                                start=(kc == 0), stop=(kc == KCH - 1))
                        if j == 3:
                            nc.vector.reciprocal(den[:, nk:nk + 1],
                                                 pso[:, 256:257])
                            nc.vector.tensor_scalar_mul(onorm[:, nk, :],
                                                        pso[:, 0:256],
                                                        den[:, nk:nk + 1])
                            onT = otp.tile([128, 2, 128], bf, tag="onT",
                                           bufs=10, name=f"onT{h - 1}_{nk}")
                            onTs[(h - 1, nk)] = onT
                            nc.sync.dma_start(out=onT, in_=onorm[:, nk, :],
                                              transpose=True)
                            if h == H and nk >= 1:
                                # final head: upproj nk-1 while nk's PV ran,
                                # covering the transpose latency
                                emit_upproj_nk(nk - 1)
                if h >= 1:
                    pts.pop(h - 1)
                if h == H:
                    emit_upproj_nk(NCK - 1)
                    wvu_d.pop(H - 1)
                    nc.sync.dma_start(
                        out=OUT.rearrange("(c p) e -> p c e", p=128)
                        [:, :, (H - 1) * HD:H * HD],
                        in_=otF)

            if loop_ctx is not None:
                loop_ctx.__exit__(None, None, None)

    nc.finalize()
    return nc


def get_nc(iters=1, stop_after="full", fake_ckv=False):
    key = (iters, stop_after, fake_ckv)
    if key not in _cache:
        _cache[key] = build_nc(iters, stop_after, fake_ckv)
    return _cache[key]


def make_in_maps(Q, K, Wq_d, W_qk, Wkv_d, Wv_u):
    from ml_dtypes import bfloat16

    Q = np.asarray(Q, dtype=np.float32)
    K = np.asarray(K, dtype=np.float32)
    weights = {
        "WQD": np.ascontiguousarray(np.asarray(Wq_d, np.float32)).astype(bfloat16),
        "WQK": np.ascontiguousarray(np.asarray(W_qk, np.float32)).astype(bfloat16),
        "WKVD": np.ascontiguousarray(np.asarray(Wkv_d, np.float32)).astype(bfloat16),
        "WVU": np.ascontiguousarray(np.asarray(Wv_u, np.float32)).astype(bfloat16),
    }
    kts = [np.ascontiguousarray(K[b].T).astype(bfloat16) for b in range(B)]
    qts = [np.ascontiguousarray(Q[b].T).astype(bfloat16) for b in range(B)]
    in_maps = []
    for c in range(NCORES):
        b, q = divmod(c, 4)
        m = dict(weights)
        m["KT"] = kts[b]
        m["QT"] = np.ascontiguousarray(qts[b][:, q * NQ:(q + 1) * NQ])
        in_maps.append(m)
    return in_maps


def kernel(Q, K, Wq_d, W_qk, Wkv_d, Wv_u):
    from concourse.bass_utils import run_bass_kernel_spmd

    nc = get_nc(1)
    in_maps = make_in_maps(Q, K, Wq_d, W_qk, Wkv_d, Wv_u)
    res = run_bass_kernel_spmd(nc, in_maps, core_ids=list(range(NCORES)))
    out = np.empty((B, N, E), dtype=np.float32)
    for c in range(NCORES):
        b, q = divmod(c, 4)
        out[b, q * NQ:(q + 1) * NQ, :] = np.asarray(
            res.results[c]["OUT"], dtype=np.float32)
    return out
